# revision 1
# baseline (speedup 1.0000x reference)
"""ArcMarginLoss distributed Trainium2 kernel (8 NeuronCores, class-sharded).

Math (equivalent to the reference, no arccos needed):
  x_hat = x / max(||x||, eps);  w_hat = w / max(||w||, eps)
  cos[i,c] = x_hat[i] . w_hat[c]
  For the label class only: m_i = cos(arccos(clip(c_i)) + M)
                                = clip(c_i)*cos(M) - sin(M)*sqrt(1-clip(c_i)^2)
  logits = S*cos except S*m_i at the label
  nll_i = logsumexp_c(logits[i]) - S*m_i
        = ln( sum_c exp(S*cos[i,c]) - exp(S*c_i) + exp(S*m_i) ) - S*m_i
  out = mean_i nll_i
S*cos is in [-16, 16] so no max-subtraction is needed for a stable sum-exp.

Distribution: classes padded 32000 -> 32768 with zero rows and sharded
contiguously, 4096 per core.  Each core computes its local sum-exp plus its
owned rows' correction terms; two [128,64] f32 AllReduces (one per row half,
the first issued mid-loop) combine
  A[i] = sum_c exp(S*cos) - exp(S*c_i) + exp(S*m_i)   (pads add exp(0)=1 each)
  B[i] = S*m_i
then every core computes mean(ln(A - 768) - B).

Implementation notes:
- matmul runs in bf16 (fp32 PE is ~3x slower: fp32_mode=LOW_HIGH); the psum
  accumulates fp32 and the label-correction path stays fp32, so the overall
  rel err stays ~3e-7.
- exp is fused on the Scalar engine: activation(Exp, scale=S/||x||_row as a
  per-partition AP, accum_out=row sums) over [128,1024] psum pairs.
- all rsqrt/sqrt are computed as exp/ln so one ACT table set serves the
  whole kernel (walrus swaps tables between Ln/Exp sets; batches minimize
  swaps).
- x rows/w classes are packed 4/8 per partition ((p a) layout) for 8-16KB
  DMA descriptors; the class permutation is harmless (softmax sums classes)
  and the row permutation is undone on the host via lab/msk layout.
- x tiles are bf16-transposed via one dma_start_transpose per group; w tiles
  via PE identity transposes (PE is idle during the W stage).
"""

import math
import sys

sys.path.insert(0, "/opt/trn_rl_repo")

import numpy as np

from concourse import bacc, bass, mybir, tile
from concourse.bass_utils import run_bass_kernel_spmd
from concourse.masks import make_identity

f32 = mybir.dt.float32
bf16 = mybir.dt.bfloat16
i32 = mybir.dt.int32

N, D, C = 8192, 512, 32000
NCORES = 8
CPAD = 32768            # padded class count (8 * 4096)
CS = CPAD // NCORES     # classes per core
P = 128                 # partitions
RT = N // P             # row tiles (64)
CT = CS // 512          # class tiles of 512 (8)
DC = D // P             # contraction chunks (4)
NPAD = float(CPAD - C)  # zero-pad classes, each contributes exp(0)=1

S_SCALE = 16.0
M_MARGIN = 0.2
EPS = 1e-7
COS_M = math.cos(M_MARGIN)
LN_SIN_M = math.log(math.sin(M_MARGIN))
SS_FLOOR = 1e-24        # max(ss, floor) emulates torch F.normalize eps=1e-12

_CACHE = {}


def _build(ncores=NCORES):
    nc = bacc.Bacc("TRN2", target_bir_lowering=False, debug=False,
                   num_devices=ncores)
    x_d = nc.dram_tensor("x", [N, D], f32, kind="ExternalInput")
    w_d = nc.dram_tensor("w", [CS, D], f32, kind="ExternalInput")
    lab_d = nc.dram_tensor("lab", [P, RT], i32, kind="ExternalInput")
    msk_d = nc.dram_tensor("msk", [P, RT], f32, kind="ExternalInput")
    out_d = nc.dram_tensor("out", [1, 1], f32, kind="ExternalOutput")

    mult = mybir.AluOpType.mult
    add = mybir.AluOpType.add
    sub = mybir.AluOpType.subtract
    amax = mybir.AluOpType.max
    amin = mybir.AluOpType.min
    Exp = mybir.ActivationFunctionType.Exp
    Ln = mybir.ActivationFunctionType.Ln
    NW = CS // P       # 32 weight chunks of 128 classes
    XG = RT // 4       # 16 x groups (512 rows each)
    WG = 4             # w groups (1024 classes each, 8 chunks)
    HB = RT // 2       # half of the row-tile columns

    with tile.TileContext(nc) as tc:
        with tc.tile_pool(name="persist", bufs=1) as persist, \
             tc.tile_pool(name="dram", bufs=1, space="DRAM") as dram, \
             tc.tile_pool(name="io", bufs=3) as io, \
             tc.tile_pool(name="big", bufs=3) as big, \
             tc.tile_pool(name="xhp", bufs=8) as xhp, \
             tc.tile_pool(name="expp", bufs=3) as expp, \
             tc.tile_pool(name="small", bufs=4) as small, \
             tc.tile_pool(name="pmm", bufs=3, space="PSUM") as pmm, \
             tc.tile_pool(name="ptp", bufs=2, space="PSUM") as ptp:

            def T(shape, name, dtype=f32):
                return persist.tile(shape, dtype, name=name)

            ones = T([P, 1], "ones")
            nc.vector.memset(ones[:], 1.0)
            lnS_c = T([P, 1], "lnS_c")
            nc.vector.memset(lnS_c[:], math.log(S_SCALE))
            lnsinM_c = T([P, 1], "lnsinM_c")
            nc.vector.memset(lnsinM_c[:], LN_SIN_M)
            identity_b = T([P, P], "identity_b", dtype=bf16)
            make_identity(nc, identity_b[:])

            labs = T([P, RT], "labs", dtype=i32)
            nc.gpsimd.dma_start(out=labs[:], in_=lab_d[:, :])
            msks = T([P, RT], "msks")
            nc.gpsimd.dma_start(out=msks[:], in_=msk_d[:, :])

            sumexp = T([P, RT], "sumexp")
            cdot = T([P, RT], "cdot")
            ctl = T([P, RT], "ctl")
            marg = T([P, RT], "marg")
            aloc = T([P, RT], "aloc")
            bloc = T([P, RT], "bloc")
            ssw_all = T([P, NW], "ssw_all")
            ssx_all = T([P, RT], "ssx_all")
            ssg_all = T([P, RT], "ssg_all")
            dotg_all = T([P, RT], "dotg_all")
            sxs_all = T([P, RT], "sxs_all")   # S / ||x_row||
            wsc_all = T([P, NW], "wsc_all")
            ar_in0 = dram.tile([P, RT], f32)
            ar_out0 = dram.tile([P, RT], f32)
            ar_in1 = dram.tile([P, RT], f32)
            ar_out1 = dram.tile([P, RT], f32)

            # bf16 transposed, normalized w (class order permuted; softmax
            # is order-invariant), one tile per 512-class ct block:
            # whT[ct][p, dc, (j%4)*128+q] covers class chunk j = ct*4+j%4.
            whT = [T([P, DC, 512], f"whT_{ct}", dtype=bf16)
                   for ct in range(CT)]

            xhb_tiles = {}
            xb_tiles = {}
            blk_starts = {1: 0, 3: 2, 7: 4, 11: 8, 15: 12}

            def emit_xload(g):
                xt = io.tile([P, 4, D], f32, name="xt", tag="xt", bufs=3)
                # row r = g*512 + p*4 + a  -> 8KB contiguous/partition
                nc.sync.dma_start(
                    out=xt[:],
                    in_=x_d[g * 4 * P:(g + 1) * 4 * P, :].rearrange(
                        "(p a) d -> p a d", a=4))
                xb = big.tile([P, 4, D], bf16, name="xb", bufs=8)
                xb_tiles[g] = xb
                nc.vector.tensor_copy(out=xb[:], in_=xt[:])
                xhb = xhp.tile([P, 16, P], bf16, name="xhb")
                xhb_tiles[g] = xhb
                nc.sync.dma_start_transpose(
                    xhb[:], xb.rearrange("p a d -> p (a d)")[:])
                for a in range(4):
                    t = g * 4 + a
                    xjk = big.tile([P, D], f32, name="junk", bufs=2)
                    nc.vector.scalar_tensor_tensor(
                        out=xjk[:], in0=xt[:, a], scalar=1.0, in1=xt[:, a],
                        op0=mult, op1=mult,
                        accum_out=ssx_all[:, t:t + 1])

            def emit_gather(g):
                # label-row gather + dot products; reads the pinned bf16
                # x cast so nothing upstream waits on these
                xb_g = xb_tiles.pop(g)
                for a in range(4):
                    t = g * 4 + a
                    wg_t = big.tile([P, D], f32, name="wg")
                    nc.gpsimd.indirect_dma_start(
                        out=wg_t[:], out_offset=None, in_=w_d[:, :],
                        in_offset=bass.IndirectOffsetOnAxis(
                            ap=labs[:, t:t + 1], axis=0))
                    gjk = big.tile([P, D], f32, name="gjunk", bufs=2)
                    nc.vector.scalar_tensor_tensor(
                        out=gjk[:], in0=wg_t[:], scalar=1.0, in1=wg_t[:],
                        op0=mult, op1=mult,
                        accum_out=ssg_all[:, t:t + 1])
                    gjk2 = big.tile([P, D], f32, name="gjunk", bufs=2)
                    nc.vector.scalar_tensor_tensor(
                        out=gjk2[:], in0=wg_t[:], scalar=1.0, in1=xb_g[:, a],
                        op0=mult, op1=mult,
                        accum_out=dotg_all[:, t:t + 1])

            def emit_sxs_batch(g):
                # x rows are random normals; ||x||^2 is never near zero, so
                # the reference's max(.,eps) clamp is inactive for x
                g_lo = blk_starts[g]
                blk = slice(g_lo * 4, (g + 1) * 4)
                nblk = (g + 1 - g_lo) * 4
                xln = small.tile([P, 16], f32, name="xln")
                nc.scalar.activation(out=xln[:, :nblk],
                                     in_=ssx_all[:, blk], func=Ln)
                nc.scalar.activation(out=sxs_all[:, blk],
                                     in_=xln[:, :nblk],
                                     func=Exp, scale=-0.5,
                                     bias=lnS_c[:, :1])

            def emit_half(h):
                # corrections + allreduce for rows of column half h
                cl = slice(h * HB, (h + 1) * HB)
                gssc = small.tile([P, HB], f32, name="gssc")
                nc.vector.tensor_scalar_max(out=gssc[:], in0=ssg_all[:, cl],
                                            scalar1=SS_FLOOR)
                glns = small.tile([P, HB], f32, name="glns")
                nc.scalar.activation(out=glns[:], in_=gssc[:], func=Ln)
                gsc = small.tile([P, HB], f32, name="gsc")
                nc.scalar.activation(out=gsc[:], in_=glns[:], func=Exp,
                                     scale=-0.5)
                cd = cdot[:, cl]
                nc.vector.tensor_tensor(out=cd, in0=dotg_all[:, cl],
                                        in1=gsc[:], op=mult)
                nc.vector.tensor_tensor(out=cd, in0=cd, in1=sxs_all[:, cl],
                                        op=mult)
                nc.vector.tensor_scalar_mul(out=cd, in0=cd,
                                            scalar1=1.0 / S_SCALE)
                nc.vector.tensor_scalar(out=ctl[:, cl], in0=cd,
                                        scalar1=(-1.0 + EPS),
                                        scalar2=(1.0 - EPS),
                                        op0=amax, op1=amin)
                negc2 = small.tile([P, HB], f32, name="negc2")
                nc.vector.scalar_tensor_tensor(out=negc2[:], in0=ctl[:, cl],
                                               scalar=-1.0, in1=ctl[:, cl],
                                               op0=mult, op1=mult)
                uu = small.tile([P, HB], f32, name="uu")
                nc.vector.tensor_scalar_add(out=uu[:], in0=negc2[:],
                                            scalar1=1.0)
                lnu = small.tile([P, HB], f32, name="lnu")
                nc.scalar.activation(out=lnu[:], in_=uu[:], func=Ln)
                sinsq = small.tile([P, HB], f32, name="sinsq")
                nc.scalar.activation(out=sinsq[:], in_=lnu[:], func=Exp,
                                     scale=0.5, bias=lnsinM_c[:, :1])
                nc.vector.scalar_tensor_tensor(out=marg[:, cl],
                                               in0=ctl[:, cl],
                                               scalar=COS_M, in1=sinsq[:],
                                               op0=mult, op1=sub)
                e1 = small.tile([P, HB], f32, name="e1")
                nc.scalar.activation(out=e1[:], in_=marg[:, cl], func=Exp,
                                     scale=S_SCALE)
                e2 = small.tile([P, HB], f32, name="e2")
                nc.scalar.activation(out=e2[:], in_=ctl[:, cl], func=Exp,
                                     scale=S_SCALE)
                d12 = small.tile([P, HB], f32, name="d12")
                nc.vector.scalar_tensor_tensor(out=d12[:], in0=e1[:],
                                               scalar=1.0, in1=e2[:],
                                               op0=mult, op1=sub)
                corr = small.tile([P, HB], f32, name="corr")
                nc.vector.tensor_tensor(out=corr[:], in0=d12[:],
                                        in1=msks[:, cl], op=mult)
                nc.vector.tensor_tensor(out=aloc[:, cl], in0=sumexp[:, cl],
                                        in1=corr[:], op=add)
                nc.vector.scalar_tensor_tensor(out=bloc[:, cl],
                                               in0=marg[:, cl],
                                               scalar=S_SCALE,
                                               in1=msks[:, cl],
                                               op0=mult, op1=mult)
                ar_i = ar_in0 if h == 0 else ar_in1
                ar_o = ar_out0 if h == 0 else ar_out1
                nc.gpsimd.dma_start(out=ar_i[:, 0:HB], in_=aloc[:, cl])
                nc.gpsimd.dma_start(out=ar_i[:, HB:2 * HB], in_=bloc[:, cl])
                nc.gpsimd.collective_compute(
                    "AllReduce", add,
                    replica_groups=[list(range(ncores))],
                    ins=[ar_i[:].opt()], outs=[ar_o[:].opt()])

            # hoist the first 4 x groups so their ssx / sxs and transposed
            # tiles are ready as soon as the W stage finishes
            for g in range(4):
                emit_xload(g)
                if g in (1, 3):
                    emit_sxs_batch(g)

            taccs_all = {}

            def emit_block_pass(tiles, ct2):
                for t in tiles:
                    if t not in taccs_all:
                        taccs_all[t] = small.tile([P, CT // 2], f32,
                                                  name="accs", bufs=36)
                for t in tiles:
                    g2, a = t // 4, t % 4
                    ps = pmm.tile([P, 1024], f32, name="ps")
                    for half in range(2):
                        ct = ct2 * 2 + half
                        for dc in range(DC):
                            nc.tensor.matmul(
                                out=ps[:, half * 512:(half + 1) * 512],
                                lhsT=xhb_tiles[g2][:, a * 4 + dc, :],
                                rhs=whT[ct][:, dc, :],
                                start=(dc == 0), stop=(dc == DC - 1))
                    ej = expp.tile([P, 1024], f32, name="ej", bufs=1)
                    nc.scalar.activation(
                        out=ej[:], in_=ps[:], func=Exp,
                        scale=sxs_all[:, t:t + 1],
                        accum_out=taccs_all[t][:, ct2:ct2 + 1])

            def emit_block_reduce(tiles):
                for t in tiles:
                    nc.vector.reduce_sum(
                        out=sumexp[:, t:t + 1],
                        in_=taccs_all.pop(t)[:, :CT // 2],
                        axis=mybir.AxisListType.X)

            # ---- stage W: load all, one batched norm, scale + PE
            #      transpose (keeps the sync DMA queue free for x) ----
            for gw in range(WG):
                wt = io.tile([P, 8, D], f32, name="wt", tag="wt", bufs=2)
                # class c = gw*1024 + p*8 + a  -> 16KB contiguous/partition
                nc.scalar.dma_start(
                    out=wt[:],
                    in_=w_d[gw * 8 * P:(gw + 1) * 8 * P, :].rearrange(
                        "(p a) d -> p a d", a=8))
                for a in range(8):
                    j = gw * 8 + a
                    wjk = big.tile([P, D], f32, name="junk", bufs=2)
                    nc.vector.scalar_tensor_tensor(
                        out=wjk[:], in0=wt[:, a], scalar=1.0, in1=wt[:, a],
                        op0=mult, op1=mult,
                        accum_out=ssw_all[:, j:j + 1])
                gsl = slice(gw * 8, (gw + 1) * 8)
                wssc = small.tile([P, 8], f32, name="wssc")
                nc.vector.tensor_scalar_max(out=wssc[:],
                                            in0=ssw_all[:, gsl],
                                            scalar1=SS_FLOOR)
                wlns = small.tile([P, 8], f32, name="wlns")
                nc.scalar.activation(out=wlns[:], in_=wssc[:], func=Ln)
                nc.scalar.activation(out=wsc_all[:, gsl], in_=wlns[:],
                                     func=Exp, scale=-0.5)
                for a in range(8):
                    j = gw * 8 + a
                    wnb = big.tile([P, D], bf16, name="wnb", bufs=3)
                    nc.vector.tensor_scalar_mul(
                        out=wnb[:], in0=wt[:, a],
                        scalar1=wsc_all[:, j:j + 1])
                    for dc in range(DC):
                        pst = ptp.tile([P, P], bf16, name="pst", tag="pst")
                        nc.tensor.transpose(
                            out=pst[:],
                            in_=wnb[:, dc * P:(dc + 1) * P],
                            identity=identity_b[:])
                        nc.vector.tensor_copy(
                            out=whT[j // 4][:, dc,
                                            (j % 4) * P:(j % 4 + 1) * P],
                            in_=pst[:])
                if gw >= 1:
                    emit_block_pass(list(range(8)), gw - 1)
            emit_block_pass(list(range(8)), 3)
            emit_block_reduce(list(range(8)))

            # ---- main loop: x loads 4 groups ahead, gathers 2 ahead,
            #      sxs batches 2 iterations ahead of their block ----
            batch_at = {5: 7, 9: 11, 13: 15}
            blk2 = {3: 2, 7: 4, 11: 8, 15: 12}
            for g in range(XG):
                if g + 4 < XG:
                    emit_xload(g + 4)
                if g == 0:
                    emit_gather(0)
                    emit_gather(1)
                if g + 2 < XG:
                    emit_gather(g + 2)
                if g in batch_at:
                    emit_sxs_batch(batch_at[g])
                if g in blk2:
                    tiles = list(range(blk2[g] * 4, (g + 1) * 4))
                    for ct2 in range(CT // 2):
                        emit_block_pass(tiles, ct2)
                    emit_block_reduce(tiles)
                if g == 11:
                    emit_half(0)
            emit_half(1)

            # ---- combine halves and reduce to the scalar mean ----
            gg = T([P, 2 * RT], "gg")
            nc.gpsimd.dma_start(out=gg[:, 0:HB], in_=ar_out0[:, 0:HB])
            nc.gpsimd.dma_start(out=gg[:, HB:RT], in_=ar_out1[:, 0:HB])
            nc.gpsimd.dma_start(out=gg[:, RT:RT + HB], in_=ar_out0[:, HB:RT])
            nc.gpsimd.dma_start(out=gg[:, RT + HB:2 * RT],
                                in_=ar_out1[:, HB:RT])

            at = T([P, RT], "at")
            nc.vector.tensor_scalar_add(out=at[:], in0=gg[:, 0:RT],
                                        scalar1=-NPAD)
            lna = T([P, RT], "lna")
            nc.scalar.activation(out=lna[:], in_=at[:], func=Ln)
            nll = T([P, RT], "nll")
            nc.vector.scalar_tensor_tensor(out=nll[:], in0=lna[:], scalar=1.0,
                                           in1=gg[:, RT:2 * RT],
                                           op0=mult, op1=sub)
            rsum = T([P, 1], "rsum")
            nc.vector.reduce_sum(out=rsum[:], in_=nll[:],
                                 axis=mybir.AxisListType.X)
            pf = ptp.tile([1, 1], f32, name="pf", tag="pst")
            nc.tensor.matmul(out=pf[:1, :1], lhsT=rsum[:, :1],
                             rhs=ones[:, :1], start=True, stop=True)
            res = T([1, 1], "res")
            nc.vector.tensor_scalar_mul(out=res[:], in0=pf[:1, :1],
                                        scalar1=1.0 / float(N))
            nc.gpsimd.dma_start(out=out_d[:, :], in_=res[:])

    nc.compile()
    return nc


def _get_nc():
    if "nc" not in _CACHE:
        _CACHE["nc"] = _build()
    return _CACHE["nc"]


def kernel(prev_output, weight, labels, **trace_kwargs):
    x = np.ascontiguousarray(prev_output, dtype=np.float32)
    w = np.ascontiguousarray(weight, dtype=np.float32)
    lab = np.asarray(labels).astype(np.int64)

    wpad = np.zeros((CPAD, D), dtype=np.float32)
    wpad[:C] = w

    in_maps = []
    for k in range(NCORES):
        lo = k * CS
        loc = (lab - lo).astype(np.int64)
        own = (loc >= 0) & (loc < CS)
        locc = np.clip(loc, 0, CS - 1).astype(np.int32)
        # row r = g*512 + p*4 + a maps to [p, t=g*4+a]
        lab2 = locc.reshape(RT // 4, P, 4).transpose(1, 0, 2).reshape(P, RT)
        msk2 = own.astype(np.float32).reshape(RT // 4, P, 4) \
                  .transpose(1, 0, 2).reshape(P, RT)
        in_maps.append({
            "x": x,
            "w": wpad[lo:lo + CS],
            "lab": np.ascontiguousarray(lab2),
            "msk": np.ascontiguousarray(msk2),
        })

    nc = _get_nc()
    res = run_bass_kernel_spmd(nc, in_maps, core_ids=list(range(NCORES)),
                               **trace_kwargs)
    if trace_kwargs:
        _CACHE["last_results"] = res
    return np.float32(res.results[0]["out"].reshape(())[()])


if __name__ == "__main__":
    rng = np.random.default_rng(0)
    x = rng.standard_normal((N, D), dtype=np.float32)
    w = rng.standard_normal((C, D), dtype=np.float32) * 0.01
    lab = rng.integers(0, C, N)
    got = kernel(x, w, lab)
    xh = x / np.maximum(np.linalg.norm(x, axis=1, keepdims=True), 1e-12)
    wh = w / np.maximum(np.linalg.norm(w, axis=1, keepdims=True), 1e-12)
    cos = (xh @ wh.T).astype(np.float64)
    th = np.arccos(np.clip(cos[np.arange(N), lab], -1 + EPS, 1 - EPS))
    ml = np.cos(th + M_MARGIN)
    logits = cos * S_SCALE
    tgt = ml * S_SCALE
    lse = np.log(np.exp(logits).sum(1) - np.exp(logits[np.arange(N), lab])
                 + np.exp(tgt))
    want = (lse - tgt).mean()
    print("got", got, "want", want, "relerr", abs(got - want) / abs(want))



# revision 10
# speedup vs baseline: 1.2853x; 1.2853x over previous
"""ArcMarginLoss distributed Trainium2 kernel (8 NeuronCores, class-sharded).

Math (equivalent to the reference, no arccos needed):
  x_hat = x / max(||x||, eps);  w_hat = w / max(||w||, eps)
  cos[i,c] = x_hat[i] . w_hat[c]
  For the label class only: m_i = cos(arccos(clip(c_i)) + M)
                                = clip(c_i)*cos(M) - sin(M)*sqrt(1-clip(c_i)^2)
  logits = S*cos except S*m_i at the label
  nll_i = logsumexp_c(logits[i]) - S*m_i
        = ln( sum_c exp(S*cos[i,c]) - exp(S*c_i) + exp(S*m_i) ) - S*m_i
  out = mean_i nll_i
S*cos is in [-16, 16] so no max-subtraction is needed for a stable sum-exp.

Distribution: classes padded 32000 -> 32768 with zero rows and sharded
contiguously, 4096 per core.  Each core computes its local sum-exp plus its
owned rows' correction terms; two [128,64] f32 AllReduces (one per row half,
the first issued mid-loop) combine
  A[i] = sum_c exp(S*cos) - exp(S*c_i) + exp(S*m_i)   (pads add exp(0)=1 each)
  B[i] = S*m_i
then every core computes mean(ln(A - 768) - B).

Implementation notes:
- The bulk cosine matmul runs in fp8e4 (e4m3) with MatmulPerfMode.DoubleRow:
  each instruction contracts K=256 (two 128-blocks) at 0.5 PE cycles/row.
  w_hat is pre-scaled by 16 so its entries (~N(0, 1/512)) sit in e4m3's
  normal range; x is cast raw (entries ~N(0,1)).  The per-row exp scale
  1/||x|| absorbs S/16 exactly (S=16).  The label-correction path stays
  fp32, so the final rel err stays ~1e-4.
- both operands are transposed in bf16 on the DMA transpose engine (the
  xbar path needs 2-byte elements, and dual-fp8 ldweights requires the
  standard two-contiguous-128-block k-tile layout), then cast bf16 -> fp8
  (x casts on GpSimd, w on DVE, balancing engine load).
- exp is fused on the Scalar engine: activation(Exp, scale=1/||x||_row as a
  per-partition AP, accum_out=row sums) over [128,2048] psum tiles (4 banks,
  double-buffered = all 8), written back in place to psum.
- all rsqrt/sqrt are computed as exp/ln so one ACT table set serves the
  whole kernel.
- label-row gather dot products run on the GpSimd engine to keep DVE free;
  x rows/w classes are packed 4/8 per partition ((p a) layout) for 8-16KB
  DMA descriptors; the class permutation is harmless (softmax sums classes)
  and the row permutation is undone on the host via lab/msk layout.
"""

import math
import sys

sys.path.insert(0, "/opt/trn_rl_repo")

import numpy as np

from concourse import bacc, bass, mybir, tile
from concourse.bass_utils import run_bass_kernel_spmd

f32 = mybir.dt.float32
f8 = mybir.dt.float8e4
bf16 = mybir.dt.bfloat16
u16 = mybir.dt.uint16
i32 = mybir.dt.int32

N, D, C = 8192, 512, 32000
NCORES = 8
CPAD = 32768            # padded class count (8 * 4096)
CS = CPAD // NCORES     # classes per core
P = 128                 # partitions
RT = N // P             # row tiles (64)
CT = CS // 512          # class tiles of 512 (8)
NPAD = float(CPAD - C)  # zero-pad classes, each contributes exp(0)=1

S_SCALE = 16.0
M_MARGIN = 0.2
EPS = 1e-7
COS_M = math.cos(M_MARGIN)
LN_SIN_M = math.log(math.sin(M_MARGIN))
SS_FLOOR = 1e-24        # max(ss, floor) emulates torch F.normalize eps=1e-12
W8 = 16.0               # w_hat fp8 pre-scale; folded into the exp row scale
LN_W8 = math.log(W8)

_CACHE = {}


def _build(ncores=NCORES):
    nc = bacc.Bacc("TRN2", target_bir_lowering=False, debug=False,
                   num_devices=ncores)
    x_d = nc.dram_tensor("x", [N, D], f32, kind="ExternalInput")
    w_d = nc.dram_tensor("w", [CS, D], f32, kind="ExternalInput")
    lab_d = nc.dram_tensor("lab", [P, RT], i32, kind="ExternalInput")
    msk_d = nc.dram_tensor("msk", [P, RT], f32, kind="ExternalInput")
    out_d = nc.dram_tensor("out", [1, 1], f32, kind="ExternalOutput")

    mult = mybir.AluOpType.mult
    add = mybir.AluOpType.add
    sub = mybir.AluOpType.subtract
    amax = mybir.AluOpType.max
    amin = mybir.AluOpType.min
    Exp = mybir.ActivationFunctionType.Exp
    Ln = mybir.ActivationFunctionType.Ln
    DR = mybir.MatmulPerfMode.DoubleRow
    NW = CS // P       # 32 weight chunks of 128 classes
    XG = RT // 4       # 16 x groups (512 rows each)
    WG = 4             # w groups (1024 classes each, 8 chunks)
    HB = RT // 2       # half of the row-tile columns

    with tile.TileContext(nc) as tc:
        with tc.tile_pool(name="persist", bufs=1) as persist, \
             tc.tile_pool(name="dram", bufs=1, space="DRAM") as dram, \
             tc.tile_pool(name="io", bufs=3) as io, \
             tc.tile_pool(name="big", bufs=3) as big, \
             tc.tile_pool(name="xhp", bufs=8) as xhp, \
             tc.tile_pool(name="small", bufs=4) as small, \
             tc.tile_pool(name="pmm", bufs=2, space="PSUM") as pmm:

            def T(shape, name, dtype=f32):
                return persist.tile(shape, dtype, name=name)

            ones = T([P, 1], "ones")
            nc.vector.memset(ones[:], 1.0)
            lnsinM_c = T([P, 1], "lnsinM_c")
            nc.vector.memset(lnsinM_c[:], LN_SIN_M)
            lnW8_c = T([P, 1], "lnW8_c")
            nc.vector.memset(lnW8_c[:], LN_W8)

            labs = T([P, RT], "labs", dtype=i32)
            nc.gpsimd.dma_start(out=labs[:], in_=lab_d[:, :])
            msks = T([P, RT], "msks")
            nc.gpsimd.dma_start(out=msks[:], in_=msk_d[:, :])

            sumexp = T([P, RT], "sumexp")
            cdot = T([P, RT], "cdot")
            ctl = T([P, RT], "ctl")
            marg = T([P, RT], "marg")
            aloc = T([P, RT], "aloc")
            bloc = T([P, RT], "bloc")
            ssw_all = T([P, NW], "ssw_all")
            ssx_all = T([P, RT], "ssx_all")
            ssg_all = T([P, RT], "ssg_all")
            dotg_all = T([P, RT], "dotg_all")
            sxs_all = T([P, RT], "sxs_all")   # 1 / ||x_row||
            wsc_all = T([P, NW], "wsc_all")   # 16 / ||w_class||
            ar_in0 = dram.tile([P, RT], f32)
            ar_out0 = dram.tile([P, RT], f32)
            ar_in1 = dram.tile([P, RT], f32)
            ar_out1 = dram.tile([P, RT], f32)

            # fp8 transposed, normalized, 16x-scaled w (class order permuted;
            # softmax is order-invariant), one tile per 512-class ct block:
            # whT8[ct][p, dc, c] = 16*w_hat[class(ct,c), dc*128 + p].
            whT8 = [T([P, 4, 512], f"whT8_{ct}", dtype=f8)
                    for ct in range(CT)]

            xh8_tiles = {}
            xb_tiles = {}
            blk_starts = {1: 0, 3: 2, 7: 4, 11: 8, 15: 12}

            def emit_xload(g):
                xt = io.tile([P, 4, D], f32, name="xt", tag="xt", bufs=3)
                # row r = g*512 + p*4 + a  -> 8KB contiguous/partition
                nc.sync.dma_start(
                    out=xt[:],
                    in_=x_d[g * 4 * P:(g + 1) * 4 * P, :].rearrange(
                        "(p a) d -> p a d", a=4))
                xb = big.tile([P, 4, D], bf16, name="xb", bufs=8)
                xb_tiles[g] = xb
                nc.vector.tensor_copy(out=xb[:], in_=xt[:])
                xhb = big.tile([P, 16, P], bf16, name="xhb", bufs=3)
                nc.sync.dma_start_transpose(
                    xhb[:], xb.rearrange("p a d -> p (a d)")[:])
                xh8 = xhp.tile([P, 16, P], f8, name="xh8")
                xh8_tiles[g] = xh8
                nc.vector.tensor_copy(out=xh8[:], in_=xhb[:])
                for a in range(4):
                    t = g * 4 + a
                    xjk = big.tile([P, D], f32, name="junk", bufs=2)
                    nc.vector.scalar_tensor_tensor(
                        out=xjk[:], in0=xt[:, a], scalar=1.0, in1=xt[:, a],
                        op0=mult, op1=mult,
                        accum_out=ssx_all[:, t:t + 1])

            def emit_gather(g):
                # label-row gather + dot products; reads the pinned bf16
                # x cast so nothing upstream waits on these
                xb_g = xb_tiles.pop(g)
                for a in range(4):
                    t = g * 4 + a
                    wg_t = big.tile([P, D], f32, name="wg")
                    nc.gpsimd.indirect_dma_start(
                        out=wg_t[:], out_offset=None, in_=w_d[:, :],
                        in_offset=bass.IndirectOffsetOnAxis(
                            ap=labs[:, t:t + 1], axis=0))
                    gjk = big.tile([P, D], f32, name="gjunk", bufs=2)
                    nc.vector.scalar_tensor_tensor(
                        out=gjk[:], in0=wg_t[:], scalar=1.0, in1=wg_t[:],
                        op0=mult, op1=mult,
                        accum_out=ssg_all[:, t:t + 1])
                    gjk2 = big.tile([P, D], f32, name="gjunk2", bufs=2)
                    nc.vector.scalar_tensor_tensor(
                        out=gjk2[:], in0=wg_t[:], scalar=1.0,
                        in1=xb_g[:, a],
                        op0=mult, op1=mult,
                        accum_out=dotg_all[:, t:t + 1])

            def emit_sxs_batch(g):
                # x rows are random normals; ||x||^2 is never near zero, so
                # the reference's max(.,eps) clamp is inactive for x
                g_lo = blk_starts[g]
                blk = slice(g_lo * 4, (g + 1) * 4)
                nblk = (g + 1 - g_lo) * 4
                xln = small.tile([P, 16], f32, name="xln")
                nc.scalar.activation(out=xln[:, :nblk],
                                     in_=ssx_all[:, blk], func=Ln)
                nc.scalar.activation(out=sxs_all[:, blk],
                                     in_=xln[:, :nblk],
                                     func=Exp, scale=-0.5)

            def emit_half(h):
                # corrections + allreduce for rows of column half h
                cl = slice(h * HB, (h + 1) * HB)
                gssc = small.tile([P, HB], f32, name="gssc")
                nc.vector.tensor_scalar_max(out=gssc[:], in0=ssg_all[:, cl],
                                            scalar1=SS_FLOOR)
                glns = small.tile([P, HB], f32, name="glns")
                nc.scalar.activation(out=glns[:], in_=gssc[:], func=Ln)
                gsc = small.tile([P, HB], f32, name="gsc")
                nc.scalar.activation(out=gsc[:], in_=glns[:], func=Exp,
                                     scale=-0.5)
                cd = cdot[:, cl]
                nc.vector.tensor_tensor(out=cd, in0=dotg_all[:, cl],
                                        in1=gsc[:], op=mult)
                nc.vector.tensor_tensor(out=cd, in0=cd, in1=sxs_all[:, cl],
                                        op=mult)
                nc.vector.tensor_scalar(out=ctl[:, cl], in0=cd,
                                        scalar1=(-1.0 + EPS),
                                        scalar2=(1.0 - EPS),
                                        op0=amax, op1=amin)
                negc2 = small.tile([P, HB], f32, name="negc2")
                nc.vector.scalar_tensor_tensor(out=negc2[:], in0=ctl[:, cl],
                                               scalar=-1.0, in1=ctl[:, cl],
                                               op0=mult, op1=mult)
                uu = small.tile([P, HB], f32, name="uu")
                nc.vector.tensor_scalar_add(out=uu[:], in0=negc2[:],
                                            scalar1=1.0)
                lnu = small.tile([P, HB], f32, name="lnu")
                nc.scalar.activation(out=lnu[:], in_=uu[:], func=Ln)
                sinsq = small.tile([P, HB], f32, name="sinsq")
                nc.scalar.activation(out=sinsq[:], in_=lnu[:], func=Exp,
                                     scale=0.5, bias=lnsinM_c[:, :1])
                nc.vector.scalar_tensor_tensor(out=marg[:, cl],
                                               in0=ctl[:, cl],
                                               scalar=COS_M, in1=sinsq[:],
                                               op0=mult, op1=sub)
                e1 = small.tile([P, HB], f32, name="e1")
                nc.scalar.activation(out=e1[:], in_=marg[:, cl], func=Exp,
                                     scale=S_SCALE)
                e2 = small.tile([P, HB], f32, name="e2")
                nc.scalar.activation(out=e2[:], in_=ctl[:, cl], func=Exp,
                                     scale=S_SCALE)
                d12 = small.tile([P, HB], f32, name="d12")
                nc.vector.scalar_tensor_tensor(out=d12[:], in0=e1[:],
                                               scalar=1.0, in1=e2[:],
                                               op0=mult, op1=sub)
                corr = small.tile([P, HB], f32, name="corr")
                nc.vector.tensor_tensor(out=corr[:], in0=d12[:],
                                        in1=msks[:, cl], op=mult)
                nc.vector.tensor_tensor(out=aloc[:, cl], in0=sumexp[:, cl],
                                        in1=corr[:], op=add)
                nc.vector.scalar_tensor_tensor(out=bloc[:, cl],
                                               in0=marg[:, cl],
                                               scalar=S_SCALE,
                                               in1=msks[:, cl],
                                               op0=mult, op1=mult)
                ar_i = ar_in0 if h == 0 else ar_in1
                ar_o = ar_out0 if h == 0 else ar_out1
                nc.gpsimd.dma_start(out=ar_i[:, 0:HB], in_=aloc[:, cl])
                nc.gpsimd.dma_start(out=ar_i[:, HB:2 * HB], in_=bloc[:, cl])
                nc.gpsimd.collective_compute(
                    "AllReduce", add,
                    replica_groups=[list(range(ncores))],
                    ins=[ar_i[:].opt()], outs=[ar_o[:].opt()])

            # hoist the first 4 x groups so their ssx / sxs and transposed
            # tiles are ready as soon as the first W half finishes
            for g in range(4):
                emit_xload(g)
                if g in (1, 3):
                    emit_sxs_batch(g)

            taccs_all = {}

            def emit_block_pass(tiles, ct4):
                for t in tiles:
                    if t not in taccs_all:
                        taccs_all[t] = small.tile([P, 2], f32,
                                                  name="accs", bufs=36)
                for t in tiles:
                    g2, a = t // 4, t % 4
                    ps = pmm.tile([P, 2048], f32, name="ps")
                    for q in range(4):
                        ct = ct4 * 4 + q
                        for b in range(2):
                            nc.tensor.matmul(
                                out=ps[:, q * 512:(q + 1) * 512],
                                lhsT=xh8_tiles[g2][:, a * 4 + 2 * b:
                                                   a * 4 + 2 * b + 2, :],
                                rhs=whT8[ct][:, 2 * b:2 * b + 2, :],
                                start=(b == 0), stop=(b == 1),
                                perf_mode=DR)
                    nc.scalar.activation(
                        out=ps[:], in_=ps[:], func=Exp,
                        scale=sxs_all[:, t:t + 1],
                        accum_out=taccs_all[t][:, ct4:ct4 + 1])

            def emit_block_reduce(tiles):
                for t in tiles:
                    acc = taccs_all.pop(t)
                    nc.vector.tensor_tensor(
                        out=sumexp[:, t:t + 1], in0=acc[:, 0:1],
                        in1=acc[:, 1:2], op=add)

            # ---- stage W: load all, one batched norm, scale to fp8, and
            #      u16-pair DMA transpose ----
            for gw in range(WG):
                wt = io.tile([P, 8, D], f32, name="wt", tag="wt", bufs=2)
                # class c = gw*1024 + p*8 + a  -> 16KB contiguous/partition
                nc.scalar.dma_start(
                    out=wt[:],
                    in_=w_d[gw * 8 * P:(gw + 1) * 8 * P, :].rearrange(
                        "(p a) d -> p a d", a=8))
                for a in range(8):
                    j = gw * 8 + a
                    wjk = big.tile([P, D], f32, name="junk", bufs=2)
                    nc.vector.scalar_tensor_tensor(
                        out=wjk[:], in0=wt[:, a], scalar=1.0, in1=wt[:, a],
                        op0=mult, op1=mult,
                        accum_out=ssw_all[:, j:j + 1])
                gsl = slice(gw * 8, (gw + 1) * 8)
                wssc = small.tile([P, 8], f32, name="wssc")
                nc.vector.tensor_scalar_max(out=wssc[:],
                                            in0=ssw_all[:, gsl],
                                            scalar1=SS_FLOOR)
                wlns = small.tile([P, 8], f32, name="wlns")
                nc.scalar.activation(out=wlns[:], in_=wssc[:], func=Ln)
                nc.scalar.activation(out=wsc_all[:, gsl], in_=wlns[:],
                                     func=Exp, scale=-0.5,
                                     bias=lnW8_c[:, :1])
                for a in range(8):
                    j = gw * 8 + a
                    wnb = big.tile([P, D], bf16, name="wnb", bufs=3)
                    nc.vector.tensor_scalar_mul(
                        out=wnb[:], in0=wt[:, a],
                        scalar1=wsc_all[:, j:j + 1])
                    whTb = big.tile([P, 4, P], bf16, name="whTb", bufs=3)
                    nc.sync.dma_start_transpose(whTb[:], wnb[:])
                    ct, jj = j // 4, j % 4
                    nc.vector.tensor_copy(
                        out=whT8[ct][:, :, jj * 128:(jj + 1) * 128],
                        in_=whTb[:])
                if gw == 1:
                    emit_block_pass(list(range(8)), 0)
            emit_block_pass(list(range(8)), 1)
            emit_block_reduce(list(range(8)))

            # ---- main loop: x loads 4 groups ahead, gathers 2 ahead,
            #      sxs batches 2 iterations ahead of their block ----
            batch_at = {5: 7, 9: 11, 13: 15}
            blk2 = {3: 2, 7: 4, 11: 8, 15: 12}
            for g in range(XG):
                if g + 4 < XG:
                    emit_xload(g + 4)
                if g == 0:
                    emit_gather(0)
                    emit_gather(1)
                if g + 2 < XG:
                    emit_gather(g + 2)
                if g in batch_at:
                    emit_sxs_batch(batch_at[g])
                if g in blk2:
                    tiles = list(range(blk2[g] * 4, (g + 1) * 4))
                    for ct4 in range(2):
                        emit_block_pass(tiles, ct4)
                    emit_block_reduce(tiles)
                if g == 11:
                    emit_half(0)
            emit_half(1)

            # ---- combine halves and reduce to the scalar mean ----
            gg = T([P, 2 * RT], "gg")
            nc.gpsimd.dma_start(out=gg[:, 0:HB], in_=ar_out0[:, 0:HB])
            nc.gpsimd.dma_start(out=gg[:, HB:RT], in_=ar_out1[:, 0:HB])
            nc.gpsimd.dma_start(out=gg[:, RT:RT + HB], in_=ar_out0[:, HB:RT])
            nc.gpsimd.dma_start(out=gg[:, RT + HB:2 * RT],
                                in_=ar_out1[:, HB:RT])

            at = T([P, RT], "at")
            nc.vector.tensor_scalar_add(out=at[:], in0=gg[:, 0:RT],
                                        scalar1=-NPAD)
            lna = T([P, RT], "lna")
            nc.scalar.activation(out=lna[:], in_=at[:], func=Ln)
            nll = T([P, RT], "nll")
            nc.vector.scalar_tensor_tensor(out=nll[:], in0=lna[:], scalar=1.0,
                                           in1=gg[:, RT:2 * RT],
                                           op0=mult, op1=sub)
            rsum = T([P, 1], "rsum")
            nc.vector.reduce_sum(out=rsum[:], in_=nll[:],
                                 axis=mybir.AxisListType.X)
            pf = pmm.tile([P, 2048], f32, name="ps")
            nc.tensor.matmul(out=pf[:1, :1], lhsT=rsum[:, :1],
                             rhs=ones[:, :1], start=True, stop=True)
            res = T([1, 1], "res")
            nc.vector.tensor_scalar_mul(out=res[:], in0=pf[:1, :1],
                                        scalar1=1.0 / float(N))
            nc.gpsimd.dma_start(out=out_d[:, :], in_=res[:])

    nc.compile()
    return nc


def _get_nc():
    if "nc" not in _CACHE:
        _CACHE["nc"] = _build()
    return _CACHE["nc"]


def kernel(prev_output, weight, labels, **trace_kwargs):
    x = np.ascontiguousarray(prev_output, dtype=np.float32)
    w = np.ascontiguousarray(weight, dtype=np.float32)
    lab = np.asarray(labels).astype(np.int64)

    wpad = np.zeros((CPAD, D), dtype=np.float32)
    wpad[:C] = w

    in_maps = []
    for k in range(NCORES):
        lo = k * CS
        loc = (lab - lo).astype(np.int64)
        own = (loc >= 0) & (loc < CS)
        locc = np.clip(loc, 0, CS - 1).astype(np.int32)
        # row r = g*512 + p*4 + a maps to [p, t=g*4+a]
        lab2 = locc.reshape(RT // 4, P, 4).transpose(1, 0, 2).reshape(P, RT)
        msk2 = own.astype(np.float32).reshape(RT // 4, P, 4) \
                  .transpose(1, 0, 2).reshape(P, RT)
        in_maps.append({
            "x": x,
            "w": wpad[lo:lo + CS],
            "lab": np.ascontiguousarray(lab2),
            "msk": np.ascontiguousarray(msk2),
        })

    nc = _get_nc()
    res = run_bass_kernel_spmd(nc, in_maps, core_ids=list(range(NCORES)),
                               **trace_kwargs)
    if trace_kwargs:
        _CACHE["last_results"] = res
    return np.float32(res.results[0]["out"].reshape(())[()])


if __name__ == "__main__":
    rng = np.random.default_rng(0)
    x = rng.standard_normal((N, D), dtype=np.float32)
    w = rng.standard_normal((C, D), dtype=np.float32) * 0.01
    lab = rng.integers(0, C, N)
    got = kernel(x, w, lab)
    xh = x / np.maximum(np.linalg.norm(x, axis=1, keepdims=True), 1e-12)
    wh = w / np.maximum(np.linalg.norm(w, axis=1, keepdims=True), 1e-12)
    cos = (xh @ wh.T).astype(np.float64)
    th = np.arccos(np.clip(cos[np.arange(N), lab], -1 + EPS, 1 - EPS))
    ml = np.cos(th + M_MARGIN)
    logits = cos * S_SCALE
    tgt = ml * S_SCALE
    lse = np.log(np.exp(logits).sum(1) - np.exp(logits[np.arange(N), lab])
                 + np.exp(tgt))
    want = (lse - tgt).mean()
    print("got", got, "want", want, "relerr", abs(got - want) / abs(want))
